# revision 23
# baseline (speedup 1.0000x reference)
"""CAM-GAT layer kernel for 8 Trainium2 NeuronCores (Bass/Tile) — v2.

Reference math (per graph of N=21 joints, F=128 feats):
    h = x @ W1                         [N, F]
    s = h @ a1 ; t = h @ a2            [N]
    e[i,j] = leaky_relu(s_i + t_j, 0.2)
    beta = softmax_j(e)
    alpha = cam * beta
    x_agg = alpha @ h
    out = elu(concat([x_agg, x], -1) @ W2_w + W2_b)

Key algebra: x_agg @ W2a = alpha @ (x @ (W1 @ W2a)) = alpha @ g, so h is
never materialized; g = x @ W12a with W12a precomputed on the host.

Sharding: pure data parallelism; each core gets B/8 = 2048 graphs
(43008 rows); weights replicated.

Per-core dataflow (supertile = 504 rows = 4 chunks x 126 rows = 24 graphs):
  xT    : PE transpose of fp32 x chunks; cast to bf16 in the PSUM->SBUF copy
  s,t   : one matmul [wa1|wa2]^T @ xT -> st [2, 504]
  e_cmp : compact attention [126, (c, jj)] = [126, 4, 21]; one matmul with
          L rows = dyn s + graph indicators, R rows = chunk delta + t-reshape
  smax  : Prelu(0.2) -> Exp (compact) -> DVE row-reduce -> reciprocal ->
          beta_cmp = E * rinv (tensor_scalar per chunk)
  at    : PE transpose beta_cmp -> [84, 126]; PE spread matmul to
          [126(j), c, 126(i)]; gate+cam via one TT against static camT
  o     : per chunk: bias (K=1 mm) + at^T @ g + xT^T @ W2b in one PSUM
  elu   : em=Exp(o), r=Relu(o) on ACT; out = min(em-1, r) on GpSimd
"""

import sys

import numpy as np

try:
    import concourse  # noqa: F401
except ImportError:  # pragma: no cover
    sys.path.insert(0, "/opt/trn_rl_repo")

import ml_dtypes
import concourse.bass as bass
import concourse.bacc as bacc
import concourse.tile as tile
from concourse import mybir

FP32 = mybir.dt.float32
BF16 = mybir.dt.bfloat16
AF = mybir.ActivationFunctionType
ALU = mybir.AluOpType

N_JOINTS = 21
F = 128
B_TOTAL = 16384
N_CORES = 8
B_CORE = B_TOTAL // N_CORES            # 2048 graphs per core
ROWS_CORE = B_CORE * N_JOINTS          # 43008 rows per core

G_CHUNK = 6                            # graphs per chunk
RC = G_CHUNK * N_JOINTS                # 126 rows per chunk
NCH = 4                                # chunks per (full) supertile
ROWS_SUPER = NCH * RC                  # 504
ST_SLAB = 8                            # supertiles per DMA slab
ROWS_SLAB = ST_SLAB * ROWS_SUPER       # 4032
CH_SLAB = ST_SLAB * NCH                # 32 chunk slots per slab
KE = 10                                # e-matmul contraction depth


def _slab_plan(rows):
    """[(slab_rows, [supertile-chunklists...]), ...]"""
    plan = []
    r = 0
    while r < rows:
        sl = min(ROWS_SLAB, rows - r)
        sts = []
        c = 0
        while c < sl:
            st = min(ROWS_SUPER, sl - c)
            chunks = []
            k = 0
            while k < st:
                chunks.append(min(RC, st - k))
                k += RC
            sts.append(chunks)
            c += st
        plan.append((sl, sts))
        r += sl
    return plan


def host_consts(cam, W1, a, W2_w, W2_b):
    """Precompute tiny replicated tensors on the host (numpy)."""
    cam = np.asarray(cam, np.float32)
    W1 = np.asarray(W1, np.float32)
    a = np.asarray(a, np.float32)
    W2_w = np.asarray(W2_w, np.float32)
    W2_b = np.asarray(W2_b, np.float32)
    bf = ml_dtypes.bfloat16

    W12a = W1 @ W2_w[:F]                     # [128,128] g-space weight
    wa12 = np.stack([W1 @ a[:F], W1 @ a[F:]], axis=1)  # [128, 2]

    ident_f = np.eye(RC, dtype=np.float32)
    ident_b = ident_f.astype(bf)

    onesl = np.zeros((1, F), np.float32)
    onesl[0, :RC] = 1.0

    blk = np.arange(RC) // N_JOINTS

    # e_cmp matmul: L [10, 126]: rows 0-3 dyn s, rows 4-9 ind(i//21==q)
    L10 = np.zeros((KE, RC), np.float32)
    for q in range(G_CHUNK):
        L10[4 + q, :] = (blk == q)
    # R [10, 4, 21]: rows 0-3 delta(c-row==c), rows 4-9 dyn t-reshape
    R10 = np.zeros((KE, NCH, N_JOINTS), np.float32)
    for c in range(NCH):
        R10[c, c, :] = 1.0

    # spread stationaries SP_c [84, 128]: SP[(c',jj), j] = d(c'==c)d(jj==j%21)
    SP = np.zeros((NCH, NCH * N_JOINTS, F), np.float32)
    for c in range(NCH):
        for j in range(RC):
            SP[c, c * N_JOINTS + (j % N_JOINTS), j] = 1.0

    # camT[j, i] = cam[i%21, j%21] * (i//21 == j//21)  (gate + cam in one)
    camT = np.zeros((RC, RC), np.float32)
    for q in range(G_CHUNK):
        s0 = q * N_JOINTS
        camT[s0:s0 + N_JOINTS, s0:s0 + N_JOINTS] = cam.T

    return {
        "w12a": W12a.astype(bf),                 # [128,128]
        "w2bb": W2_w[F:].astype(bf),             # [128,128]
        "wa12": wa12.astype(bf),                 # [128,2]
        "w2brow": W2_b.reshape(1, F).astype(bf),  # [1,128]
        "identf": ident_f,                       # [126,128] f32
        "identb": ident_b,                       # [126,128] bf16
        "onesl": onesl.astype(bf),               # [1,128]
        "l10": L10.astype(bf),                   # [10,126]
        "r10": R10.astype(bf),                   # [10,4,21]
        "sp": SP.astype(bf),                     # [4,84,128]
        "camt": camT.astype(bf),                 # [126,126]
    }


CONST_SPECS = {
    "w12a": ([F, F], BF16),
    "w2bb": ([F, F], BF16),
    "wa12": ([F, 2], BF16),
    "w2brow": ([1, F], BF16),
    "identf": ([RC, RC], FP32),
    "identb": ([RC, RC], BF16),
    "onesl": ([1, F], BF16),
    "l10": ([KE, RC], BF16),
    "r10": ([KE, NCH, N_JOINTS], BF16),
    "sp": ([NCH, NCH * N_JOINTS, F], BF16),
    "camt": ([RC, RC], BF16),
}


def build_program(rows=ROWS_CORE):
    nc = bacc.Bacc("TRN2", target_bir_lowering=False, debug=False,
                   enable_asserts=False)
    x_d = nc.dram_tensor("x", [rows, F], FP32, kind="ExternalInput").ap()
    out_d = nc.dram_tensor("out", [rows, F], FP32, kind="ExternalOutput").ap()
    cst = {k: nc.dram_tensor(k, shape, dt, kind="ExternalInput").ap()
           for k, (shape, dt) in CONST_SPECS.items()}
    with tile.TileContext(nc) as tc:
        _body(tc, x_d, out_d, cst, rows)
    nc.compile()
    return nc


def _bcast_c(ap, n):
    """Insert a stride-0 dim after the partition dim: [P, X] -> [P, n, X]."""
    p, rest = ap.ap[0], list(ap.ap[1:])
    assert len(rest) == 1
    return bass.AP(ap.tensor, ap.offset, [p, [0, n], rest[0]])


def _body(tc, x_d, out_d, cst, rows):
    from contextlib import ExitStack
    nc = tc.nc
    plan = _slab_plan(rows)

    with ExitStack() as ctx:
        # ---- pools ----
        cpool = ctx.enter_context(tc.tile_pool(name="consts", bufs=1))
        pxin = ctx.enter_context(tc.tile_pool(name="xslab", bufs=2))
        pout = ctx.enter_context(tc.tile_pool(name="oslab", bufs=2))
        pxt = ctx.enter_context(tc.tile_pool(name="xt", bufs=2))
        pst = ctx.enter_context(tc.tile_pool(name="stsb", bufs=2))
        pu = ctx.enter_context(tc.tile_pool(name="ucmp", bufs=2))
        pe_ = ctx.enter_context(tc.tile_pool(name="ecmp", bufs=2))
        psc = ctx.enter_context(tc.tile_pool(name="scal", bufs=2))
        pac = ctx.enter_context(tc.tile_pool(name="acmp", bufs=2))
        pat2 = ctx.enter_context(tc.tile_pool(name="atc", bufs=2))
        pat = ctx.enter_context(tc.tile_pool(name="atbd", bufs=2))
        pg = ctx.enter_context(tc.tile_pool(name="gsb", bufs=2))
        pem = ctx.enter_context(tc.tile_pool(name="embuf", bufs=2))
        pr = ctx.enter_context(tc.tile_pool(name="rbuf", bufs=2))

        ps_xt = ctx.enter_context(tc.tile_pool(name="ps_xt", bufs=1, space="PSUM"))
        ps_st = ctx.enter_context(tc.tile_pool(name="ps_st", bufs=1, space="PSUM"))
        ps_e = ctx.enter_context(tc.tile_pool(name="ps_e", bufs=1, space="PSUM"))
        ps_at1 = ctx.enter_context(tc.tile_pool(name="ps_at1", bufs=1, space="PSUM"))
        ps_at2 = ctx.enter_context(tc.tile_pool(name="ps_at2", bufs=1, space="PSUM"))
        ps_g = ctx.enter_context(tc.tile_pool(name="ps_g", bufs=1, space="PSUM"))
        ps_o = ctx.enter_context(tc.tile_pool(name="ps_o", bufs=2, space="PSUM"))

        # ---- load constants ----
        w12a = cpool.tile([F, F], BF16, tag="w12a")
        w2bb = cpool.tile([F, F], BF16, tag="w2bb")
        wa12 = cpool.tile([F, 2], BF16, tag="wa12")
        w2brow = cpool.tile([1, F], BF16, tag="w2brow")
        identf = cpool.tile([RC, RC], FP32, tag="identf")
        identb = cpool.tile([RC, RC], BF16, tag="identb")
        onesl = cpool.tile([1, F], BF16, tag="onesl")
        sp = cpool.tile([NCH * N_JOINTS, NCH, F], BF16, tag="sp")
        camt = cpool.tile([RC, RC], BF16, tag="camt")
        for name, t in (("w12a", w12a), ("w2bb", w2bb), ("wa12", wa12),
                        ("w2brow", w2brow), ("identf", identf),
                        ("identb", identb), ("onesl", onesl), ("camt", camt)):
            nc.sync.dma_start(t[:], cst[name][:])
        nc.sync.dma_start(sp[:], cst["sp"].rearrange("c k f -> k c f"))

        # L/R e-matmul tiles (even/odd persistent)
        LRs = []
        for par in ("ev", "od"):
            Lt = cpool.tile([KE, RC], BF16, tag=f"L_{par}")
            Rt = cpool.tile([KE, NCH, N_JOINTS], BF16, tag=f"R_{par}")
            nc.sync.dma_start(Lt[:], cst["l10"][:])
            nc.sync.dma_start(Rt[:], cst["r10"][:])
            LRs.append((Lt, Rt))

        r0 = 0
        sti = 0
        for slab_rows, sts in plan:
            nfull = slab_rows // RC          # full 126-row chunk slots
            rem = slab_rows - nfull * RC     # tail rows in slot `nfull`

            x_sl = pxin.tile([RC, CH_SLAB, F], FP32, tag="x_sl")
            o_sl = pout.tile([RC, CH_SLAB, F], FP32, tag="o_sl")
            if nfull:
                nc.sync.dma_start(
                    x_sl[:, 0:nfull, :],
                    x_d[r0:r0 + nfull * RC, :].rearrange(
                        "(c i) f -> i c f", i=RC))
            if rem:
                nc.gpsimd.memset(x_sl[:, nfull, :], 0.0)
                nc.sync.dma_start(
                    x_sl[0:rem, nfull, :],
                    x_d[r0 + nfull * RC:r0 + slab_rows, :])

            c0 = 0
            for chunks in sts:
                nch = len(chunks)
                Lt, Rt = LRs[sti % 2]
                sti += 1

                # -- transpose x chunks (fp32): xt_ps [128, c, 128] --
                xt_ps = ps_xt.tile([F, NCH, F], FP32, tag="xt_ps")
                for c in range(nch):
                    nc.tensor.transpose(xt_ps[:, c, 0:RC],
                                        x_sl[:, c0 + c, :], identf[:])
                xt = pxt.tile([F, NCH, F], BF16, tag="xt")
                if sti <= 2:
                    nc.gpsimd.memset(xt[:, :, RC:F], 0.0)
                nh = (nch + 1) // 2
                nc.vector.tensor_copy(xt[:, 0:nh, 0:RC],
                                      xt_ps[:, 0:nh, 0:RC])
                if nch > nh:
                    nc.scalar.copy(xt[:, nh:nch, 0:RC],
                                   xt_ps[:, nh:nch, 0:RC])

                # -- s/t: st [2, (c, j)] --
                st_ps = ps_st.tile([2, NCH, RC], FP32, tag="st_ps")
                nc.tensor.matmul(st_ps[:, 0:nch, :], wa12[:],
                                 xt[:, 0:nch, 0:RC], start=True, stop=True)
                st_sb = pst.tile([2, NCH, RC], BF16, tag="st_sb")
                nc.vector.tensor_copy(st_sb[:, 0:nch, :], st_ps[:, 0:nch, :])

                # -- scatter s into L rows 0-3, t-reshape into R rows 4-9 --
                nc.gpsimd.dma_start(Lt[0:nch, :], st_sb[0:1, 0:nch, :])
                for q in range(G_CHUNK):
                    nc.gpsimd.dma_start(
                        Rt[4 + q:5 + q, 0:nch, :],
                        st_sb[1:2, 0:nch,
                              q * N_JOINTS:(q + 1) * N_JOINTS])

                # -- compact e: e[i, (c, jj)] = s[c,i] + t[mate] --
                e_ps = ps_e.tile([RC, NCH, N_JOINTS], FP32, tag="e_ps")
                nc.tensor.matmul(e_ps[:, 0:nch, :], Lt[:], Rt[:, 0:nch, :],
                                 start=True, stop=True)

                # -- softmax (compact): prelu -> exp -> reduce -> recip --
                u = pu.tile([RC, NCH, N_JOINTS], FP32, tag="u")
                nc.scalar.activation(u[:, 0:nch, :], e_ps[:, 0:nch, :],
                                     AF.Prelu, alpha=0.2)
                E = pe_.tile([RC, NCH, N_JOINTS], BF16, tag="E")
                nc.scalar.activation(E[:, 0:nch, :], u[:, 0:nch, :], AF.Exp)
                rowsum = psc.tile([RC, NCH], FP32, tag="rowsum")
                nc.vector.tensor_reduce(rowsum[:, 0:nch], E[:, 0:nch, :],
                                        mybir.AxisListType.X, ALU.add)
                rinv = psc.tile([RC, NCH], FP32, tag="rinv")
                nc.vector.reciprocal(rinv[:, 0:nch], rowsum[:, 0:nch])

                # -- beta_cmp = E * rinv --
                acmp = pac.tile([RC, NCH, N_JOINTS], BF16, tag="acmp")
                for c in range(nch):
                    nc.vector.tensor_scalar(
                        acmp[:, c, :], E[:, c, :], rinv[:, c:c + 1], None,
                        ALU.mult)

                # -- transpose compact beta: [84, 126] --
                at1_ps = ps_at1.tile([NCH * N_JOINTS, F], BF16, tag="at1_ps")
                nc.tensor.transpose(at1_ps[0:nch * N_JOINTS, 0:RC],
                                    acmp[:, 0:nch, :], identb[:])
                atc = pat2.tile([NCH * N_JOINTS, RC], BF16, tag="atc")
                nc.vector.tensor_copy(atc[0:nch * N_JOINTS, :],
                                      at1_ps[0:nch * N_JOINTS, 0:RC])

                # -- spread to block-diag (ungated), then gate*cam --
                at2_ps = ps_at2.tile([F, NCH, RC], FP32, tag="at2_ps")
                for c in range(nch):
                    nc.tensor.matmul(at2_ps[:, c, :],
                                     sp[0:nch * N_JOINTS, c, :],
                                     atc[0:nch * N_JOINTS, :],
                                     start=True, stop=True)
                at = pat.tile([RC, NCH, F], BF16, tag="at")
                if sti <= 2:
                    nc.gpsimd.memset(at[:, :, RC:F], 0.0)
                nc.vector.tensor_tensor(
                    at[:, 0:nch, 0:RC], at2_ps[0:RC, 0:nch, :],
                    _bcast_c(camt[:], nch), ALU.mult)

                # -- g = x @ W12a (row-major, bf16) --
                g_ps = ps_g.tile([F, NCH, F], FP32, tag="g_ps")
                for c in range(nch):
                    nc.tensor.matmul(g_ps[:, c, :], xt[:, c, :], w12a[:],
                                     start=True, stop=True)
                g = pg.tile([F, NCH, F], BF16, tag="g")
                nc.scalar.copy(g[:, 0:nch, :], g_ps[:, 0:nch, :])

                # -- o = bias + at^T @ g + x @ W2b --
                o_ps = ps_o.tile([F, NCH, F], FP32, tag="o_ps")
                for c in range(nch):
                    nc.tensor.matmul(o_ps[:, c, :], onesl[:], w2brow[:],
                                     start=True, stop=False)
                    nc.tensor.matmul(o_ps[:, c, :], at[:, c, :],
                                     g[0:RC, c, :], start=False, stop=False)
                    nc.tensor.matmul(o_ps[:, c, :], xt[:, c, :], w2bb[:],
                                     start=False, stop=True)

                # -- elu: em=exp(o), r=relu(o) (ACT); min(em-1, r) (POOL) --
                em = pem.tile([RC, NCH, F], BF16, tag="em")
                nc.scalar.activation(em[:, 0:nch, :], o_ps[0:RC, 0:nch, :],
                                     AF.Exp)
                rr = pr.tile([RC, NCH, F], BF16, tag="rr")
                nc.scalar.activation(rr[:, 0:nch, :], o_ps[0:RC, 0:nch, :],
                                     AF.Relu)
                nc.vector.scalar_tensor_tensor(
                    o_sl[:, c0:c0 + nch, :], em[:, 0:nch, :], -1.0,
                    rr[:, 0:nch, :], op0=ALU.add, op1=ALU.min)

                c0 += nch

            # -- store slab --
            if nfull:
                nc.sync.dma_start(
                    out_d[r0:r0 + nfull * RC, :].rearrange(
                        "(c i) f -> i c f", i=RC),
                    o_sl[:, 0:nfull, :])
            if rem:
                nc.sync.dma_start(
                    out_d[r0 + nfull * RC:r0 + slab_rows, :],
                    o_sl[0:rem, nfull, :])
            r0 += slab_rows


# ---------------------------------------------------------------------------
_PROG_CACHE = {}


def _get_program(rows):
    if rows not in _PROG_CACHE:
        _PROG_CACHE[rows] = build_program(rows)
    return _PROG_CACHE[rows]


def kernel(x, cam, W1, a, W2_w, W2_b):
    from concourse.bass_utils import run_bass_kernel_spmd

    x = np.ascontiguousarray(np.asarray(x, np.float32))
    consts = host_consts(cam, W1, a, W2_w, W2_b)
    nc = _get_program(ROWS_CORE)

    in_maps = []
    for core in range(N_CORES):
        m = {"x": x[core * ROWS_CORE:(core + 1) * ROWS_CORE]}
        m.update(consts)
        in_maps.append(m)
    res = run_bass_kernel_spmd(nc, in_maps, list(range(N_CORES)))
    out = np.concatenate([res.results[i]["out"] for i in range(N_CORES)], axis=0)
    return out.astype(np.float32)


# revision 41
# speedup vs baseline: 1.4463x; 1.4463x over previous
"""CAM-GAT layer kernel for 8 Trainium2 NeuronCores (Bass/Tile) — v2.

Reference math (per graph of N=21 joints, F=128 feats):
    h = x @ W1                         [N, F]
    s = h @ a1 ; t = h @ a2            [N]
    e[i,j] = leaky_relu(s_i + t_j, 0.2)
    beta = softmax_j(e)
    alpha = cam * beta
    x_agg = alpha @ h
    out = elu(concat([x_agg, x], -1) @ W2_w + W2_b)

Key algebra: x_agg @ W2a = alpha @ (x @ (W1 @ W2a)) = alpha @ g, so h is
never materialized; g = x @ W12a with W12a precomputed on the host.

Sharding: pure data parallelism; each core gets B/8 = 2048 graphs
(43008 rows); weights replicated.

Per-core dataflow (supertile = 504 rows = 4 chunks x 126 rows = 24 graphs):
  xT    : PE transpose of fp32 x chunks; cast to bf16 in the PSUM->SBUF copy
  s,t   : one matmul [wa1|wa2]^T @ xT -> st [2, 504]
  e_cmp : compact attention [126, (c, jj)] = [126, 4, 21]; one matmul with
          L rows = dyn s + graph indicators, R rows = chunk delta + t-reshape
  smax  : Prelu(0.2) -> Exp (compact) -> DVE row-reduce -> reciprocal ->
          beta_cmp = E * rinv (tensor_scalar per chunk)
  at    : PE transpose beta_cmp -> [84, 126]; PE spread matmul to
          [126(j), c, 126(i)]; gate+cam via one TT against static camT
  o     : per chunk: bias (K=1 mm) + at^T @ g + xT^T @ W2b in one PSUM
  elu   : em=Exp(o), r=Relu(o) on ACT; out = min(em-1, r) on GpSimd
"""

import sys

import numpy as np

try:
    import concourse  # noqa: F401
except ImportError:  # pragma: no cover
    sys.path.insert(0, "/opt/trn_rl_repo")

import ml_dtypes
import concourse.bass as bass
import concourse.bacc as bacc
import concourse.tile as tile
from concourse import mybir

FP32 = mybir.dt.float32
BF16 = mybir.dt.bfloat16
AF = mybir.ActivationFunctionType
ALU = mybir.AluOpType

N_JOINTS = 21
F = 128
B_TOTAL = 16384
N_CORES = 8
B_CORE = B_TOTAL // N_CORES            # 2048 graphs per core
ROWS_CORE = B_CORE * N_JOINTS          # 43008 rows per core

G_CHUNK = 6                            # graphs per chunk
RC = G_CHUNK * N_JOINTS                # 126 rows per chunk
NCH = 4                                # chunks per (full) supertile
ROWS_SUPER = NCH * RC                  # 504
ST_SLAB = 8                            # supertiles per DMA slab
ROWS_SLAB = ST_SLAB * ROWS_SUPER       # 4032
CH_SLAB = ST_SLAB * NCH                # 32 chunk slots per slab
KE = 10                                # e-matmul contraction depth


def _slab_plan(rows):
    """[(slab_rows, [supertile-chunklists...]), ...]"""
    plan = []
    r = 0
    while r < rows:
        sl = min(ROWS_SLAB, rows - r)
        sts = []
        c = 0
        while c < sl:
            st = min(ROWS_SUPER, sl - c)
            chunks = []
            k = 0
            while k < st:
                chunks.append(min(RC, st - k))
                k += RC
            sts.append(chunks)
            c += st
        plan.append((sl, sts))
        r += sl
    return plan


def host_consts(cam, W1, a, W2_w, W2_b):
    """Precompute tiny replicated tensors on the host (numpy)."""
    cam = np.asarray(cam, np.float32)
    W1 = np.asarray(W1, np.float32)
    a = np.asarray(a, np.float32)
    W2_w = np.asarray(W2_w, np.float32)
    W2_b = np.asarray(W2_b, np.float32)
    bf = ml_dtypes.bfloat16

    W12a = W1 @ W2_w[:F]                     # [128,128] g-space weight
    wa12 = np.stack([W1 @ a[:F], W1 @ a[F:]], axis=1)  # [128, 2]

    ident_f = np.eye(RC, dtype=np.float32)
    ident_b = ident_f.astype(bf)

    blk = np.arange(RC) // N_JOINTS

    # e_s matmul rhs: wa1 broadcast over the 21 mate columns
    wa1ones = np.tile((W1 @ a[:F])[:, None], (1, N_JOINTS))  # [128, 21]
    # e_t stationary: lind[q, i] = ind(i//21 == q), padded to 128 cols
    lind = np.zeros((G_CHUNK, F), np.float32)
    for q in range(G_CHUNK):
        lind[q, :RC] = (blk == q)

    # spread stationaries SP_c [84, 128]: SP[(c',jj), j] = d(c'==c)d(jj==j%21)
    SP = np.zeros((NCH, NCH * N_JOINTS, F), np.float32)
    for c in range(NCH):
        for j in range(RC):
            SP[c, c * N_JOINTS + (j % N_JOINTS), j] = 1.0

    # camT[j, i] = cam[i%21, j%21] * (i//21 == j//21)  (gate + cam in one)
    camT = np.zeros((RC, RC), np.float32)
    for q in range(G_CHUNK):
        s0 = q * N_JOINTS
        camT[s0:s0 + N_JOINTS, s0:s0 + N_JOINTS] = cam.T

    atpad = np.zeros((2, NCH, F), np.float32)
    atpad[0] = 1.0

    return {
        "atpad": atpad.astype(bf),               # [2,4,128]
        "w12a": W12a.astype(bf),                 # [128,128]
        "w2bb": W2_w[F:].astype(bf),             # [128,128]
        "wa2c": wa12[:, 1:2].astype(bf),         # [128,1]
        "w2brow": W2_b.reshape(1, F).astype(bf),  # [1,128]
        "identf": ident_f,                       # [126,126] f32
        "identb": ident_b,                       # [126,126] bf16
        "wa1ones": wa1ones.astype(bf),           # [128,21]
        "lind": lind.astype(bf),                 # [6,128]
        "sp": SP.astype(bf),                     # [4,84,128]
        "camt": camT.astype(bf),                 # [126,126]
    }


CONST_SPECS = {
    "atpad": ([2, NCH, F], BF16),
    "w12a": ([F, F], BF16),
    "w2bb": ([F, F], BF16),
    "wa2c": ([F, 1], BF16),
    "w2brow": ([1, F], BF16),
    "identf": ([RC, RC], FP32),
    "identb": ([RC, RC], BF16),
    "wa1ones": ([F, N_JOINTS], BF16),
    "lind": ([G_CHUNK, F], BF16),
    "sp": ([NCH, NCH * N_JOINTS, F], BF16),
    "camt": ([RC, RC], BF16),
}


def build_program(rows=ROWS_CORE):
    nc = bacc.Bacc("TRN2", target_bir_lowering=False, debug=False,
                   enable_asserts=False)
    x_d = nc.dram_tensor("x", [rows, F], FP32, kind="ExternalInput").ap()
    out_d = nc.dram_tensor("out", [rows, F], FP32, kind="ExternalOutput").ap()
    cst = {k: nc.dram_tensor(k, shape, dt, kind="ExternalInput").ap()
           for k, (shape, dt) in CONST_SPECS.items()}
    with tile.TileContext(nc) as tc:
        _body(tc, x_d, out_d, cst, rows)
    nc.compile()
    return nc


def _bcast_c(ap, n):
    """Insert a stride-0 dim after the partition dim: [P, X] -> [P, n, X]."""
    p, rest = ap.ap[0], list(ap.ap[1:])
    assert len(rest) == 1
    return bass.AP(ap.tensor, ap.offset, [p, [0, n], rest[0]])


def _perm_qcj(xt, nch):
    """View xt [F, c, j=21q+jj] as [F, (q:6, c:nch, jj:21)]."""
    ap = xt[:, 0:nch, 0:RC]
    return bass.AP(ap.tensor, ap.offset,
                   [ap.ap[0], [N_JOINTS, G_CHUNK], [F, nch], [1, N_JOINTS]])


def _body(tc, x_d, out_d, cst, rows):
    from contextlib import ExitStack
    nc = tc.nc
    plan = _slab_plan(rows)

    with ExitStack() as ctx:
        # ---- pools ----
        cpool = ctx.enter_context(tc.tile_pool(name="consts", bufs=1))
        pxin = ctx.enter_context(tc.tile_pool(name="xslab", bufs=2))
        pout = ctx.enter_context(tc.tile_pool(name="oslab", bufs=2))
        pxt = ctx.enter_context(tc.tile_pool(name="xt", bufs=2))
        pst = ctx.enter_context(tc.tile_pool(name="stsb", bufs=2))
        pu = ctx.enter_context(tc.tile_pool(name="ucmp", bufs=2))
        pe_ = ctx.enter_context(tc.tile_pool(name="ecmp", bufs=2))
        psc = ctx.enter_context(tc.tile_pool(name="scal", bufs=2))
        pac = ctx.enter_context(tc.tile_pool(name="acmp", bufs=2))
        pat2 = ctx.enter_context(tc.tile_pool(name="atc", bufs=2))
        pat = ctx.enter_context(tc.tile_pool(name="atbd", bufs=2))
        pg = ctx.enter_context(tc.tile_pool(name="gsb", bufs=2))
        pem = ctx.enter_context(tc.tile_pool(name="embuf", bufs=2))
        pr = ctx.enter_context(tc.tile_pool(name="rbuf", bufs=2))

        ps_xt = ctx.enter_context(tc.tile_pool(name="ps_xt", bufs=1, space="PSUM"))
        ps_st = ctx.enter_context(tc.tile_pool(name="ps_st", bufs=1, space="PSUM"))
        ps_e = ctx.enter_context(tc.tile_pool(name="ps_e", bufs=1, space="PSUM"))
        ps_at1 = ctx.enter_context(tc.tile_pool(name="ps_at1", bufs=1, space="PSUM"))
        ps_at2 = ctx.enter_context(tc.tile_pool(name="ps_at2", bufs=1, space="PSUM"))
        ps_g = ctx.enter_context(tc.tile_pool(name="ps_g", bufs=1, space="PSUM"))
        ps_o = ctx.enter_context(tc.tile_pool(name="ps_o", bufs=2, space="PSUM"))

        # ---- load constants ----
        w12a = cpool.tile([F, F], BF16, tag="w12a")
        w2bb = cpool.tile([F, F], BF16, tag="w2bb")
        wa2c = cpool.tile([F, 1], BF16, tag="wa2c")
        w2brow = cpool.tile([1, F], BF16, tag="w2brow")
        identf = cpool.tile([RC, RC], FP32, tag="identf")
        identb = cpool.tile([RC, RC], BF16, tag="identb")
        wa1ones = cpool.tile([F, N_JOINTS], BF16, tag="wa1ones")
        lind = cpool.tile([G_CHUNK, F], BF16, tag="lind")
        sp = cpool.tile([NCH * N_JOINTS, NCH, F], BF16, tag="sp")
        camt = cpool.tile([RC, RC], BF16, tag="camt")
        for name, t in (("w12a", w12a), ("w2bb", w2bb), ("wa2c", wa2c),
                        ("w2brow", w2brow), ("identf", identf),
                        ("identb", identb), ("wa1ones", wa1ones),
                        ("lind", lind), ("camt", camt)):
            nc.sync.dma_start(t[:], cst[name][:])
        nc.sync.dma_start(sp[:], cst["sp"].rearrange("c k f -> k c f"))

        # t-reshape tiles [q:6, c:4, jj:21] (even/odd persistent)
        r3_ev = cpool.tile([G_CHUNK, NCH, N_JOINTS], BF16, tag="R3_ev")
        r3_od = cpool.tile([G_CHUNK, NCH, N_JOINTS], BF16, tag="R3_od")
        R3s = [r3_ev, r3_od]

        r0 = 0
        sti = 0
        for slab_rows, sts in plan:
            nfull = slab_rows // RC          # full 126-row chunk slots
            rem = slab_rows - nfull * RC     # tail rows in slot `nfull`

            x_sl = pxin.tile([RC, CH_SLAB, F], FP32, tag="x_sl")
            o_sl = pout.tile([RC, CH_SLAB, F], FP32, tag="o_sl")
            if nfull:
                nc.sync.dma_start(
                    x_sl[:, 0:nfull, :],
                    x_d[r0:r0 + nfull * RC, :].rearrange(
                        "(c i) f -> i c f", i=RC))
            if rem:
                nc.gpsimd.memset(x_sl[:, nfull, :], 0.0)
                nc.sync.dma_start(
                    x_sl[0:rem, nfull, :],
                    x_d[r0 + nfull * RC:r0 + slab_rows, :])

            c0 = 0
            for chunks in sts:
                nch = len(chunks)
                R3 = R3s[sti % 2]
                sti += 1

                # -- transpose x chunks (fp32): xt_ps [128, c, 128] --
                xt_ps = ps_xt.tile([F, NCH, F], FP32, tag="xt_ps")
                for c in range(nch):
                    nc.tensor.transpose(xt_ps[:, c, 0:RC],
                                        x_sl[:, c0 + c, :], identf[:])
                xt = pxt.tile([F, NCH, F], BF16, tag="xt")
                if sti <= 2:
                    nc.gpsimd.memset(xt[:, :, RC:F], 0.0)
                if nch < NCH:
                    nc.gpsimd.memset(xt[:, nch:NCH, 0:RC], 0.0)
                nh = (nch + 1) // 2
                nc.vector.tensor_copy(xt[:, 0:nh, 0:RC],
                                      xt_ps[:, 0:nh, 0:RC])
                if nch > nh:
                    nc.scalar.copy(xt[:, nh:nch, 0:RC],
                                   xt_ps[:, nh:nch, 0:RC])

                # -- t in (q, c, jj) order: t[mate] = wa2 . x_row --
                st_ps = ps_st.tile([1, G_CHUNK, NCH, N_JOINTS], FP32,
                                   tag="st_ps")
                nc.tensor.matmul(st_ps[:], wa2c[:], _perm_qcj(xt, NCH),
                                 start=True, stop=True)
                st_sb = pst.tile([1, G_CHUNK, NCH, N_JOINTS], BF16,
                                 tag="st_sb")
                nc.vector.tensor_copy(st_sb[:], st_ps[:])
                # partition-scatter [1, (q, c, jj)] -> [q, c, jj]
                flat = st_sb[:]
                nc.gpsimd.dma_start(
                    R3[:],
                    bass.AP(flat.tensor, flat.offset,
                            [flat.ap[0], [1, G_CHUNK * NCH * N_JOINTS]]))

                # -- compact e: e[i, (c, jj)] = s_i + t[mate] --
                # s-part: xt_c^T @ (wa1 (x) ones21); t-part: lind^T @ R3_c
                e_ps = ps_e.tile([F, NCH, N_JOINTS], FP32, tag="e_ps")
                for c in range(nch):
                    nc.tensor.matmul(e_ps[:, c, :], xt[:, c, :], wa1ones[:],
                                     start=True, stop=False)
                    nc.tensor.matmul(e_ps[:, c, :], lind[:], R3[:, c, :],
                                     start=False, stop=True)

                # -- softmax (compact): prelu -> exp -> reduce -> recip --
                u = pu.tile([RC, NCH, N_JOINTS], FP32, tag="u")
                nc.scalar.activation(u[:, 0:nch, :], e_ps[0:RC, 0:nch, :],
                                     AF.Prelu, alpha=0.2)
                E = pe_.tile([RC, NCH, N_JOINTS], BF16, tag="E")
                nc.scalar.activation(E[:, 0:nch, :], u[:, 0:nch, :], AF.Exp)
                rowsum = psc.tile([RC, NCH], FP32, tag="rowsum")
                nc.vector.tensor_reduce(rowsum[:, 0:nch], E[:, 0:nch, :],
                                        mybir.AxisListType.X, ALU.add)
                rinv = psc.tile([RC, NCH], FP32, tag="rinv")
                nc.vector.reciprocal(rinv[:, 0:nch], rowsum[:, 0:nch])

                # -- beta_cmp = E * rinv --
                acmp = pac.tile([RC, NCH, N_JOINTS], BF16, tag="acmp")
                for c in range(nch):
                    nc.vector.tensor_scalar(
                        acmp[:, c, :], E[:, c, :], rinv[:, c:c + 1], None,
                        ALU.mult)

                # -- transpose compact beta: [84, 126] --
                at1_ps = ps_at1.tile([NCH * N_JOINTS, F], BF16, tag="at1_ps")
                nc.tensor.transpose(at1_ps[0:nch * N_JOINTS, 0:RC],
                                    acmp[:, 0:nch, :], identb[:])
                atc = pat2.tile([NCH * N_JOINTS, RC], BF16, tag="atc")
                nc.vector.tensor_copy(atc[0:nch * N_JOINTS, :],
                                      at1_ps[0:nch * N_JOINTS, 0:RC])

                # -- spread to block-diag (ungated), then gate*cam --
                at2_ps = ps_at2.tile([F, NCH, RC], FP32, tag="at2_ps")
                for c in range(nch):
                    nc.tensor.matmul(at2_ps[:, c, :],
                                     sp[0:nch * N_JOINTS, c, :],
                                     atc[0:nch * N_JOINTS, :],
                                     start=True, stop=True)
                at = pat.tile([F, NCH, F], BF16, tag="at")
                if sti <= 2:
                    # one-time: junk cols 0, bias row (126) = ones, row 127 = 0
                    nc.gpsimd.memset(at[0:RC, :, RC:F], 0.0)
                    nc.gpsimd.dma_start(at[RC:F, :, :], cst["atpad"][:])
                nc.vector.tensor_tensor(
                    at[0:RC, 0:nch, 0:RC], at2_ps[0:RC, 0:nch, :],
                    _bcast_c(camt[:], nch), ALU.mult)

                # -- g = x @ W12a (row-major, bf16); row 126 = W2_b --
                g_ps = ps_g.tile([F, NCH, F], FP32, tag="g_ps")
                for c in range(nch):
                    nc.tensor.matmul(g_ps[:, c, :], xt[:, c, :], w12a[:],
                                     start=True, stop=True)
                g = pg.tile([F, NCH, F], BF16, tag="g")
                if sti <= 2:
                    nc.sync.dma_start(g[RC:RC + 1, :, :],
                                      _bcast_c(cst["w2brow"][:], NCH))
                nc.scalar.copy(g[0:RC, 0:nch, :], g_ps[0:RC, 0:nch, :])

                # -- o = at^T @ [g; W2_b] + x @ W2b  (bias via K=127) --
                o_ps = ps_o.tile([F, NCH, F], FP32, tag="o_ps")
                for c in range(nch):
                    nc.tensor.matmul(o_ps[:, c, :], at[0:RC + 1, c, :],
                                     g[0:RC + 1, c, :], start=True,
                                     stop=False)
                    nc.tensor.matmul(o_ps[:, c, :], xt[:, c, :], w2bb[:],
                                     start=False, stop=True)

                # -- elu: em=exp(o), r=relu(o) (ACT); min(em-1, r) (POOL) --
                em = pem.tile([RC, NCH, F], BF16, tag="em")
                nc.scalar.activation(em[:, 0:nch, :], o_ps[0:RC, 0:nch, :],
                                     AF.Exp)
                rr = pr.tile([RC, NCH, F], BF16, tag="rr")
                nc.scalar.activation(rr[:, 0:nch, :], o_ps[0:RC, 0:nch, :],
                                     AF.Relu)
                nc.vector.scalar_tensor_tensor(
                    o_sl[:, c0:c0 + nch, :], em[:, 0:nch, :], -1.0,
                    rr[:, 0:nch, :], op0=ALU.add, op1=ALU.min)

                c0 += nch

            # -- store slab --
            if nfull:
                nc.sync.dma_start(
                    out_d[r0:r0 + nfull * RC, :].rearrange(
                        "(c i) f -> i c f", i=RC),
                    o_sl[:, 0:nfull, :])
            if rem:
                nc.sync.dma_start(
                    out_d[r0 + nfull * RC:r0 + slab_rows, :],
                    o_sl[0:rem, nfull, :])
            r0 += slab_rows


# ---------------------------------------------------------------------------
_PROG_CACHE = {}


def _get_program(rows):
    if rows not in _PROG_CACHE:
        _PROG_CACHE[rows] = build_program(rows)
    return _PROG_CACHE[rows]


def kernel(x, cam, W1, a, W2_w, W2_b):
    from concourse.bass_utils import run_bass_kernel_spmd

    x = np.ascontiguousarray(np.asarray(x, np.float32))
    consts = host_consts(cam, W1, a, W2_w, W2_b)
    nc = _get_program(ROWS_CORE)

    in_maps = []
    for core in range(N_CORES):
        m = {"x": x[core * ROWS_CORE:(core + 1) * ROWS_CORE]}
        m.update(consts)
        in_maps.append(m)
    res = run_bass_kernel_spmd(nc, in_maps, list(range(N_CORES)))
    out = np.concatenate([res.results[i]["out"] for i in range(N_CORES)], axis=0)
    return out.astype(np.float32)


# revision 42
# speedup vs baseline: 1.7315x; 1.1972x over previous
"""CAM-GAT layer kernel for 8 Trainium2 NeuronCores (Bass/Tile) — v2.

Reference math (per graph of N=21 joints, F=128 feats):
    h = x @ W1                         [N, F]
    s = h @ a1 ; t = h @ a2            [N]
    e[i,j] = leaky_relu(s_i + t_j, 0.2)
    beta = softmax_j(e)
    alpha = cam * beta
    x_agg = alpha @ h
    out = elu(concat([x_agg, x], -1) @ W2_w + W2_b)

Key algebra: x_agg @ W2a = alpha @ (x @ (W1 @ W2a)) = alpha @ g, so h is
never materialized; g = x @ W12a with W12a precomputed on the host.

Sharding: pure data parallelism; each core gets B/8 = 2048 graphs
(43008 rows); weights replicated.

Per-core dataflow (supertile = 504 rows = 4 chunks x 126 rows = 24 graphs):
  xT    : PE transpose of fp32 x chunks; cast to bf16 in the PSUM->SBUF copy
  s,t   : one matmul [wa1|wa2]^T @ xT -> st [2, 504]
  e_cmp : compact attention [126, (c, jj)] = [126, 4, 21]; one matmul with
          L rows = dyn s + graph indicators, R rows = chunk delta + t-reshape
  smax  : Prelu(0.2) -> Exp (compact) -> DVE row-reduce -> reciprocal ->
          beta_cmp = E * rinv (tensor_scalar per chunk)
  at    : PE transpose beta_cmp -> [84, 126]; PE spread matmul to
          [126(j), c, 126(i)]; gate+cam via one TT against static camT
  o     : per chunk: bias (K=1 mm) + at^T @ g + xT^T @ W2b in one PSUM
  elu   : em=Exp(o), r=Relu(o) on ACT; out = min(em-1, r) on GpSimd
"""

import sys

import numpy as np

try:
    import concourse  # noqa: F401
except ImportError:  # pragma: no cover
    sys.path.insert(0, "/opt/trn_rl_repo")

import ml_dtypes
import concourse.bass as bass
import concourse.bacc as bacc
import concourse.tile as tile
from concourse import mybir

FP32 = mybir.dt.float32
BF16 = mybir.dt.bfloat16
AF = mybir.ActivationFunctionType
ALU = mybir.AluOpType

N_JOINTS = 21
F = 128
B_TOTAL = 16384
N_CORES = 8
B_CORE = B_TOTAL // N_CORES            # 2048 graphs per core
ROWS_CORE = B_CORE * N_JOINTS          # 43008 rows per core

G_CHUNK = 6                            # graphs per chunk
RC = G_CHUNK * N_JOINTS                # 126 rows per chunk
NCH = 4                                # chunks per (full) supertile
ROWS_SUPER = NCH * RC                  # 504
ST_SLAB = 8                            # supertiles per DMA slab
ROWS_SLAB = ST_SLAB * ROWS_SUPER       # 4032
CH_SLAB = ST_SLAB * NCH                # 32 chunk slots per slab
KE = 10                                # e-matmul contraction depth


def _slab_plan(rows):
    """[(slab_rows, [supertile-chunklists...]), ...]"""
    plan = []
    r = 0
    while r < rows:
        sl = min(ROWS_SLAB, rows - r)
        sts = []
        c = 0
        while c < sl:
            st = min(ROWS_SUPER, sl - c)
            chunks = []
            k = 0
            while k < st:
                chunks.append(min(RC, st - k))
                k += RC
            sts.append(chunks)
            c += st
        plan.append((sl, sts))
        r += sl
    return plan


def host_consts(cam, W1, a, W2_w, W2_b):
    """Precompute tiny replicated tensors on the host (numpy)."""
    cam = np.asarray(cam, np.float32)
    W1 = np.asarray(W1, np.float32)
    a = np.asarray(a, np.float32)
    W2_w = np.asarray(W2_w, np.float32)
    W2_b = np.asarray(W2_b, np.float32)
    bf = ml_dtypes.bfloat16

    W12a = W1 @ W2_w[:F]                     # [128,128] g-space weight
    wa12 = np.stack([W1 @ a[:F], W1 @ a[F:]], axis=1)  # [128, 2]

    ident_f = np.eye(RC, dtype=np.float32)
    ident_b = ident_f.astype(bf)

    blk = np.arange(RC) // N_JOINTS

    # e_s matmul rhs: wa1 broadcast over the 21 mate columns
    wa1ones = np.tile((W1 @ a[:F])[:, None], (1, N_JOINTS))  # [128, 21]
    # e_t stationary: lind[q, i] = ind(i//21 == q), padded to 128 cols
    lind = np.zeros((G_CHUNK, F), np.float32)
    for q in range(G_CHUNK):
        lind[q, :RC] = (blk == q)

    # spread stationaries SP_c [84, 128]: SP[(c',jj), j] = d(c'==c)d(jj==j%21)
    SP = np.zeros((NCH, NCH * N_JOINTS, F), np.float32)
    for c in range(NCH):
        for j in range(RC):
            SP[c, c * N_JOINTS + (j % N_JOINTS), j] = 1.0

    # camT[j, i] = cam[i%21, j%21] * (i//21 == j//21)  (gate + cam in one)
    camT = np.zeros((RC, RC), np.float32)
    for q in range(G_CHUNK):
        s0 = q * N_JOINTS
        camT[s0:s0 + N_JOINTS, s0:s0 + N_JOINTS] = cam.T

    atpad = np.zeros((2, NCH, F), np.float32)
    atpad[0] = 1.0

    return {
        "atpad": atpad.astype(bf),               # [2,4,128]
        "w12a": W12a.astype(bf),                 # [128,128]
        "w2bb": W2_w[F:].astype(bf),             # [128,128]
        "wa2c": wa12[:, 1:2].astype(bf),         # [128,1]
        "w2brow": W2_b.reshape(1, F).astype(bf),  # [1,128]
        "identf": ident_f,                       # [126,126] f32
        "identb": ident_b,                       # [126,126] bf16
        "wa1ones": wa1ones.astype(bf),           # [128,21]
        "lind": lind.astype(bf),                 # [6,128]
        "sp": SP.astype(bf),                     # [4,84,128]
        "camt": camT.astype(bf),                 # [126,126]
    }


CONST_SPECS = {
    "atpad": ([2, NCH, F], BF16),
    "w12a": ([F, F], BF16),
    "w2bb": ([F, F], BF16),
    "wa2c": ([F, 1], BF16),
    "w2brow": ([1, F], BF16),
    "identf": ([RC, RC], FP32),
    "identb": ([RC, RC], BF16),
    "wa1ones": ([F, N_JOINTS], BF16),
    "lind": ([G_CHUNK, F], BF16),
    "sp": ([NCH, NCH * N_JOINTS, F], BF16),
    "camt": ([RC, RC], BF16),
}


def build_program(rows=ROWS_CORE):
    nc = bacc.Bacc("TRN2", target_bir_lowering=False, debug=False,
                   enable_asserts=False)
    x_d = nc.dram_tensor("x", [rows, F], FP32, kind="ExternalInput").ap()
    out_d = nc.dram_tensor("out", [rows, F], FP32, kind="ExternalOutput").ap()
    cst = {k: nc.dram_tensor(k, shape, dt, kind="ExternalInput").ap()
           for k, (shape, dt) in CONST_SPECS.items()}
    with tile.TileContext(nc) as tc:
        _body(tc, x_d, out_d, cst, rows)
    nc.compile()
    return nc


def _bcast_c(ap, n):
    """Insert a stride-0 dim after the partition dim: [P, X] -> [P, n, X]."""
    p, rest = ap.ap[0], list(ap.ap[1:])
    assert len(rest) == 1
    return bass.AP(ap.tensor, ap.offset, [p, [0, n], rest[0]])


def _perm_qcj(xt, nch):
    """View xt [F, c, j=21q+jj] as [F, (q:6, c:nch, jj:21)]."""
    ap = xt[:, 0:nch, 0:RC]
    return bass.AP(ap.tensor, ap.offset,
                   [ap.ap[0], [N_JOINTS, G_CHUNK], [F, nch], [1, N_JOINTS]])


def _body(tc, x_d, out_d, cst, rows):
    from contextlib import ExitStack
    nc = tc.nc
    plan = _slab_plan(rows)

    with ExitStack() as ctx:
        # ---- pools ----
        cpool = ctx.enter_context(tc.tile_pool(name="consts", bufs=1))
        pxin = ctx.enter_context(tc.tile_pool(name="xslab", bufs=2))
        pout = ctx.enter_context(tc.tile_pool(name="oslab", bufs=2))
        pxt = ctx.enter_context(tc.tile_pool(name="xt", bufs=3))
        pst = ctx.enter_context(tc.tile_pool(name="stsb", bufs=3))
        pu = ctx.enter_context(tc.tile_pool(name="ucmp", bufs=3))
        pe_ = ctx.enter_context(tc.tile_pool(name="ecmp", bufs=3))
        psc = ctx.enter_context(tc.tile_pool(name="scal", bufs=3))
        pac = ctx.enter_context(tc.tile_pool(name="acmp", bufs=3))
        pat2 = ctx.enter_context(tc.tile_pool(name="atc", bufs=3))
        pat = ctx.enter_context(tc.tile_pool(name="atbd", bufs=3))
        pg = ctx.enter_context(tc.tile_pool(name="gsb", bufs=3))
        pem = ctx.enter_context(tc.tile_pool(name="embuf", bufs=3))
        pr = ctx.enter_context(tc.tile_pool(name="rbuf", bufs=3))

        ps_xt = ctx.enter_context(tc.tile_pool(name="ps_xt", bufs=1, space="PSUM"))
        ps_st = ctx.enter_context(tc.tile_pool(name="ps_st", bufs=1, space="PSUM"))
        ps_e = ctx.enter_context(tc.tile_pool(name="ps_e", bufs=1, space="PSUM"))
        ps_at1 = ctx.enter_context(tc.tile_pool(name="ps_at1", bufs=1, space="PSUM"))
        ps_at2 = ctx.enter_context(tc.tile_pool(name="ps_at2", bufs=1, space="PSUM"))
        ps_g = ctx.enter_context(tc.tile_pool(name="ps_g", bufs=1, space="PSUM"))
        ps_o = ctx.enter_context(tc.tile_pool(name="ps_o", bufs=2, space="PSUM"))

        # ---- load constants ----
        w12a = cpool.tile([F, F], BF16, tag="w12a")
        w2bb = cpool.tile([F, F], BF16, tag="w2bb")
        wa2c = cpool.tile([F, 1], BF16, tag="wa2c")
        w2brow = cpool.tile([1, F], BF16, tag="w2brow")
        identf = cpool.tile([RC, RC], FP32, tag="identf")
        identb = cpool.tile([RC, RC], BF16, tag="identb")
        wa1ones = cpool.tile([F, N_JOINTS], BF16, tag="wa1ones")
        lind = cpool.tile([G_CHUNK, F], BF16, tag="lind")
        sp = cpool.tile([NCH * N_JOINTS, NCH, F], BF16, tag="sp")
        camt = cpool.tile([RC, RC], BF16, tag="camt")
        for name, t in (("w12a", w12a), ("w2bb", w2bb), ("wa2c", wa2c),
                        ("w2brow", w2brow), ("identf", identf),
                        ("identb", identb), ("wa1ones", wa1ones),
                        ("lind", lind), ("camt", camt)):
            nc.sync.dma_start(t[:], cst[name][:])
        nc.sync.dma_start(sp[:], cst["sp"].rearrange("c k f -> k c f"))

        # t-reshape tiles [q:6, c:4, jj:21] (even/odd persistent)
        r3_0 = cpool.tile([G_CHUNK, NCH, N_JOINTS], BF16, tag="R3_0")
        r3_1 = cpool.tile([G_CHUNK, NCH, N_JOINTS], BF16, tag="R3_1")
        r3_2 = cpool.tile([G_CHUNK, NCH, N_JOINTS], BF16, tag="R3_2")
        r3_3 = cpool.tile([G_CHUNK, NCH, N_JOINTS], BF16, tag="R3_3")
        R3s = [r3_0, r3_1, r3_2, r3_3]

        r0 = 0
        sti = 0
        for slab_rows, sts in plan:
            nfull = slab_rows // RC          # full 126-row chunk slots
            rem = slab_rows - nfull * RC     # tail rows in slot `nfull`

            x_sl = pxin.tile([RC, CH_SLAB, F], FP32, tag="x_sl")
            o_sl = pout.tile([RC, CH_SLAB, F], FP32, tag="o_sl")
            if nfull:
                nc.sync.dma_start(
                    x_sl[:, 0:nfull, :],
                    x_d[r0:r0 + nfull * RC, :].rearrange(
                        "(c i) f -> i c f", i=RC))
            if rem:
                nc.gpsimd.memset(x_sl[:, nfull, :], 0.0)
                nc.sync.dma_start(
                    x_sl[0:rem, nfull, :],
                    x_d[r0 + nfull * RC:r0 + slab_rows, :])

            c0 = 0
            for chunks in sts:
                nch = len(chunks)
                R3 = R3s[sti % 4]
                sti += 1

                # -- transpose x chunks (fp32): xt_ps [128, c, 128] --
                xt_ps = ps_xt.tile([F, NCH, F], FP32, tag="xt_ps")
                for c in range(nch):
                    nc.tensor.transpose(xt_ps[:, c, 0:RC],
                                        x_sl[:, c0 + c, :], identf[:])
                xt = pxt.tile([F, NCH, F], BF16, tag="xt")
                if sti <= 3:
                    nc.gpsimd.memset(xt[:, :, RC:F], 0.0)
                if nch < NCH:
                    nc.gpsimd.memset(xt[:, nch:NCH, 0:RC], 0.0)
                nh = (nch + 1) // 2
                nc.vector.tensor_copy(xt[:, 0:nh, 0:RC],
                                      xt_ps[:, 0:nh, 0:RC])
                if nch > nh:
                    nc.scalar.copy(xt[:, nh:nch, 0:RC],
                                   xt_ps[:, nh:nch, 0:RC])

                # -- t in (q, c, jj) order: t[mate] = wa2 . x_row --
                st_ps = ps_st.tile([1, G_CHUNK, NCH, N_JOINTS], FP32,
                                   tag="st_ps")
                nc.tensor.matmul(st_ps[:], wa2c[:], _perm_qcj(xt, NCH),
                                 start=True, stop=True)
                st_sb = pst.tile([1, G_CHUNK, NCH, N_JOINTS], BF16,
                                 tag="st_sb")
                nc.vector.tensor_copy(st_sb[:], st_ps[:])
                # partition-scatter [1, (q, c, jj)] -> [q, c, jj]
                flat = st_sb[:]
                nc.gpsimd.dma_start(
                    R3[:],
                    bass.AP(flat.tensor, flat.offset,
                            [flat.ap[0], [1, G_CHUNK * NCH * N_JOINTS]]))

                # -- compact e: e[i, (c, jj)] = s_i + t[mate] --
                # s-part: xt_c^T @ (wa1 (x) ones21); t-part: lind^T @ R3_c
                e_ps = ps_e.tile([F, NCH, N_JOINTS], FP32, tag="e_ps")
                for c in range(nch):
                    nc.tensor.matmul(e_ps[:, c, :], xt[:, c, :], wa1ones[:],
                                     start=True, stop=False)
                    nc.tensor.matmul(e_ps[:, c, :], lind[:], R3[:, c, :],
                                     start=False, stop=True)

                # -- softmax (compact): prelu -> exp -> reduce -> recip --
                u = pu.tile([RC, NCH, N_JOINTS], FP32, tag="u")
                nc.scalar.activation(u[:, 0:nch, :], e_ps[0:RC, 0:nch, :],
                                     AF.Prelu, alpha=0.2)
                E = pe_.tile([RC, NCH, N_JOINTS], BF16, tag="E")
                nc.scalar.activation(E[:, 0:nch, :], u[:, 0:nch, :], AF.Exp)
                rowsum = psc.tile([RC, NCH], FP32, tag="rowsum")
                nc.vector.tensor_reduce(rowsum[:, 0:nch], E[:, 0:nch, :],
                                        mybir.AxisListType.X, ALU.add)
                rinv = psc.tile([RC, NCH], FP32, tag="rinv")
                nc.vector.reciprocal(rinv[:, 0:nch], rowsum[:, 0:nch])

                # -- beta_cmp = E * rinv --
                acmp = pac.tile([RC, NCH, N_JOINTS], BF16, tag="acmp")
                for c in range(nch):
                    nc.vector.tensor_scalar(
                        acmp[:, c, :], E[:, c, :], rinv[:, c:c + 1], None,
                        ALU.mult)

                # -- transpose compact beta: [84, 126] --
                at1_ps = ps_at1.tile([NCH * N_JOINTS, F], BF16, tag="at1_ps")
                nc.tensor.transpose(at1_ps[0:nch * N_JOINTS, 0:RC],
                                    acmp[:, 0:nch, :], identb[:])
                atc = pat2.tile([NCH * N_JOINTS, RC], BF16, tag="atc")
                nc.vector.tensor_copy(atc[0:nch * N_JOINTS, :],
                                      at1_ps[0:nch * N_JOINTS, 0:RC])

                # -- spread to block-diag (ungated), then gate*cam --
                at2_ps = ps_at2.tile([F, NCH, RC], FP32, tag="at2_ps")
                for c in range(nch):
                    nc.tensor.matmul(at2_ps[:, c, :],
                                     sp[0:nch * N_JOINTS, c, :],
                                     atc[0:nch * N_JOINTS, :],
                                     start=True, stop=True)
                at = pat.tile([F, NCH, F], BF16, tag="at")
                if sti <= 3:
                    # one-time: junk cols 0, bias row (126) = ones, row 127 = 0
                    nc.gpsimd.memset(at[0:RC, :, RC:F], 0.0)
                    nc.gpsimd.dma_start(at[RC:F, :, :], cst["atpad"][:])
                nc.vector.tensor_tensor(
                    at[0:RC, 0:nch, 0:RC], at2_ps[0:RC, 0:nch, :],
                    _bcast_c(camt[:], nch), ALU.mult)

                # -- g = x @ W12a (row-major, bf16); row 126 = W2_b --
                g_ps = ps_g.tile([F, NCH, F], FP32, tag="g_ps")
                for c in range(nch):
                    nc.tensor.matmul(g_ps[:, c, :], xt[:, c, :], w12a[:],
                                     start=True, stop=True)
                g = pg.tile([F, NCH, F], BF16, tag="g")
                if sti <= 3:
                    nc.sync.dma_start(g[RC:RC + 1, :, :],
                                      _bcast_c(cst["w2brow"][:], NCH))
                nc.scalar.copy(g[0:RC, 0:nch, :], g_ps[0:RC, 0:nch, :])

                # -- o = at^T @ [g; W2_b] + x @ W2b  (bias via K=127) --
                o_ps = ps_o.tile([F, NCH, F], FP32, tag="o_ps")
                for c in range(nch):
                    nc.tensor.matmul(o_ps[:, c, :], at[0:RC + 1, c, :],
                                     g[0:RC + 1, c, :], start=True,
                                     stop=False)
                    nc.tensor.matmul(o_ps[:, c, :], xt[:, c, :], w2bb[:],
                                     start=False, stop=True)

                # -- elu: em=exp(o), r=relu(o) (ACT); min(em-1, r) (POOL) --
                em = pem.tile([RC, NCH, F], BF16, tag="em")
                nc.scalar.activation(em[:, 0:nch, :], o_ps[0:RC, 0:nch, :],
                                     AF.Exp)
                rr = pr.tile([RC, NCH, F], BF16, tag="rr")
                nc.scalar.activation(rr[:, 0:nch, :], o_ps[0:RC, 0:nch, :],
                                     AF.Relu)
                nc.vector.scalar_tensor_tensor(
                    o_sl[:, c0:c0 + nch, :], em[:, 0:nch, :], -1.0,
                    rr[:, 0:nch, :], op0=ALU.add, op1=ALU.min)

                c0 += nch

            # -- store slab --
            if nfull:
                nc.sync.dma_start(
                    out_d[r0:r0 + nfull * RC, :].rearrange(
                        "(c i) f -> i c f", i=RC),
                    o_sl[:, 0:nfull, :])
            if rem:
                nc.sync.dma_start(
                    out_d[r0 + nfull * RC:r0 + slab_rows, :],
                    o_sl[0:rem, nfull, :])
            r0 += slab_rows


# ---------------------------------------------------------------------------
_PROG_CACHE = {}


def _get_program(rows):
    if rows not in _PROG_CACHE:
        _PROG_CACHE[rows] = build_program(rows)
    return _PROG_CACHE[rows]


def kernel(x, cam, W1, a, W2_w, W2_b):
    from concourse.bass_utils import run_bass_kernel_spmd

    x = np.ascontiguousarray(np.asarray(x, np.float32))
    consts = host_consts(cam, W1, a, W2_w, W2_b)
    nc = _get_program(ROWS_CORE)

    in_maps = []
    for core in range(N_CORES):
        m = {"x": x[core * ROWS_CORE:(core + 1) * ROWS_CORE]}
        m.update(consts)
        in_maps.append(m)
    res = run_bass_kernel_spmd(nc, in_maps, list(range(N_CORES)))
    out = np.concatenate([res.results[i]["out"] for i in range(N_CORES)], axis=0)
    return out.astype(np.float32)


# revision 46
# speedup vs baseline: 1.7668x; 1.0204x over previous
"""CAM-GAT layer kernel for 8 Trainium2 NeuronCores (Bass/Tile) — v2.

Reference math (per graph of N=21 joints, F=128 feats):
    h = x @ W1                         [N, F]
    s = h @ a1 ; t = h @ a2            [N]
    e[i,j] = leaky_relu(s_i + t_j, 0.2)
    beta = softmax_j(e)
    alpha = cam * beta
    x_agg = alpha @ h
    out = elu(concat([x_agg, x], -1) @ W2_w + W2_b)

Key algebra: x_agg @ W2a = alpha @ (x @ (W1 @ W2a)) = alpha @ g, so h is
never materialized; g = x @ W12a with W12a precomputed on the host.

Sharding: pure data parallelism; each core gets B/8 = 2048 graphs
(43008 rows); weights replicated.

Per-core dataflow (supertile = 504 rows = 4 chunks x 126 rows = 24 graphs):
  xT    : PE transpose of fp32 x chunks; cast to bf16 in the PSUM->SBUF copy
  s,t   : one matmul [wa1|wa2]^T @ xT -> st [2, 504]
  e_cmp : compact attention [126, (c, jj)] = [126, 4, 21]; one matmul with
          L rows = dyn s + graph indicators, R rows = chunk delta + t-reshape
  smax  : Prelu(0.2) -> Exp (compact) -> DVE row-reduce -> reciprocal ->
          beta_cmp = E * rinv (tensor_scalar per chunk)
  at    : PE transpose beta_cmp -> [84, 126]; PE spread matmul to
          [126(j), c, 126(i)]; gate+cam via one TT against static camT
  o     : per chunk: bias (K=1 mm) + at^T @ g + xT^T @ W2b in one PSUM
  elu   : em=Exp(o), r=Relu(o) on ACT; out = min(em-1, r) on GpSimd
"""

import sys

import numpy as np

try:
    import concourse  # noqa: F401
except ImportError:  # pragma: no cover
    sys.path.insert(0, "/opt/trn_rl_repo")

import ml_dtypes
import concourse.bass as bass
import concourse.bacc as bacc
import concourse.tile as tile
from concourse import mybir

FP32 = mybir.dt.float32
BF16 = mybir.dt.bfloat16
AF = mybir.ActivationFunctionType
ALU = mybir.AluOpType

N_JOINTS = 21
F = 128
B_TOTAL = 16384
N_CORES = 8
B_CORE = B_TOTAL // N_CORES            # 2048 graphs per core
ROWS_CORE = B_CORE * N_JOINTS          # 43008 rows per core

G_CHUNK = 6                            # graphs per chunk
RC = G_CHUNK * N_JOINTS                # 126 rows per chunk
NCH = 4                                # chunks per (full) supertile
ROWS_SUPER = NCH * RC                  # 504
ST_SLAB = 8                            # supertiles per DMA slab
ROWS_SLAB = ST_SLAB * ROWS_SUPER       # 4032
CH_SLAB = ST_SLAB * NCH                # 32 chunk slots per slab
KE = 10                                # e-matmul contraction depth


def _slab_plan(rows):
    """[(slab_rows, [supertile-chunklists...]), ...]"""
    plan = []
    r = 0
    while r < rows:
        sl = min(ROWS_SLAB, rows - r)
        sts = []
        c = 0
        while c < sl:
            st = min(ROWS_SUPER, sl - c)
            chunks = []
            k = 0
            while k < st:
                chunks.append(min(RC, st - k))
                k += RC
            sts.append(chunks)
            c += st
        plan.append((sl, sts))
        r += sl
    return plan


def host_consts(cam, W1, a, W2_w, W2_b):
    """Precompute tiny replicated tensors on the host (numpy)."""
    cam = np.asarray(cam, np.float32)
    W1 = np.asarray(W1, np.float32)
    a = np.asarray(a, np.float32)
    W2_w = np.asarray(W2_w, np.float32)
    W2_b = np.asarray(W2_b, np.float32)
    bf = ml_dtypes.bfloat16

    W12a = W1 @ W2_w[:F]                     # [128,128] g-space weight
    wa12 = np.stack([W1 @ a[:F], W1 @ a[F:]], axis=1)  # [128, 2]

    ident_f = np.eye(RC, dtype=np.float32)
    ident_b = ident_f.astype(bf)

    blk = np.arange(RC) // N_JOINTS

    # e_s matmul rhs: wa1 broadcast over the 21 mate columns
    wa1ones = np.tile((W1 @ a[:F])[:, None], (1, N_JOINTS))  # [128, 21]
    # e_t stationary: lind[q, i] = ind(i//21 == q), padded to 128 cols
    lind = np.zeros((G_CHUNK, F), np.float32)
    for q in range(G_CHUNK):
        lind[q, :RC] = (blk == q)

    # spread stationaries SP_c [84, 128]: SP[(c',jj), j] = d(c'==c)d(jj==j%21)
    SP = np.zeros((NCH, NCH * N_JOINTS, F), np.float32)
    for c in range(NCH):
        for j in range(RC):
            SP[c, c * N_JOINTS + (j % N_JOINTS), j] = 1.0

    # camT[j, i] = cam[i%21, j%21] * (i//21 == j//21)  (gate + cam in one)
    camT = np.zeros((RC, RC), np.float32)
    for q in range(G_CHUNK):
        s0 = q * N_JOINTS
        camT[s0:s0 + N_JOINTS, s0:s0 + N_JOINTS] = cam.T

    atpad = np.zeros((2, NCH, F), np.float32)
    atpad[0] = 1.0

    return {
        "atpad": atpad.astype(bf),               # [2,4,128]
        "w12a": W12a.astype(bf),                 # [128,128]
        "w2bb": W2_w[F:].astype(bf),             # [128,128]
        "wa2c": wa12[:, 1:2].astype(bf),         # [128,1]
        "w2brow": W2_b.reshape(1, F).astype(bf),  # [1,128]
        "identf": ident_f,                       # [126,126] f32
        "identb": ident_b,                       # [126,126] bf16
        "wa1ones": wa1ones.astype(bf),           # [128,21]
        "lind": lind.astype(bf),                 # [6,128]
        "sp": SP.astype(bf),                     # [4,84,128]
        "camt": camT.astype(bf),                 # [126,126]
    }


CONST_SPECS = {
    "atpad": ([2, NCH, F], BF16),
    "w12a": ([F, F], BF16),
    "w2bb": ([F, F], BF16),
    "wa2c": ([F, 1], BF16),
    "w2brow": ([1, F], BF16),
    "identf": ([RC, RC], FP32),
    "identb": ([RC, RC], BF16),
    "wa1ones": ([F, N_JOINTS], BF16),
    "lind": ([G_CHUNK, F], BF16),
    "sp": ([NCH, NCH * N_JOINTS, F], BF16),
    "camt": ([RC, RC], BF16),
}


def build_program(rows=ROWS_CORE):
    nc = bacc.Bacc("TRN2", target_bir_lowering=False, debug=False,
                   enable_asserts=False)
    x_d = nc.dram_tensor("x", [rows, F], FP32, kind="ExternalInput").ap()
    out_d = nc.dram_tensor("out", [rows, F], FP32, kind="ExternalOutput").ap()
    cst = {k: nc.dram_tensor(k, shape, dt, kind="ExternalInput").ap()
           for k, (shape, dt) in CONST_SPECS.items()}
    with tile.TileContext(nc) as tc:
        _body(tc, x_d, out_d, cst, rows)
    nc.compile()
    return nc


def _bcast_c(ap, n):
    """Insert a stride-0 dim after the partition dim: [P, X] -> [P, n, X]."""
    p, rest = ap.ap[0], list(ap.ap[1:])
    assert len(rest) == 1
    return bass.AP(ap.tensor, ap.offset, [p, [0, n], rest[0]])


def _perm_qcj(xt_sl, c0):
    """View xt_sl chunks [c0, c0+4) [F, c, j=21q+jj] as [F, (q, c, jj)]."""
    ap = xt_sl[:, c0:c0 + NCH, 0:RC]
    return bass.AP(ap.tensor, ap.offset,
                   [ap.ap[0], [N_JOINTS, G_CHUNK], [F, NCH], [1, N_JOINTS]])


def _body(tc, x_d, out_d, cst, rows):
    from contextlib import ExitStack
    nc = tc.nc
    plan = _slab_plan(rows)

    with ExitStack() as ctx:
        # ---- pools ----
        cpool = ctx.enter_context(tc.tile_pool(name="consts", bufs=1))
        pxin = ctx.enter_context(tc.tile_pool(name="xslab", bufs=2))
        pout = ctx.enter_context(tc.tile_pool(name="oslab", bufs=2))
        pxt = ctx.enter_context(tc.tile_pool(name="xt", bufs=2))
        pst = ctx.enter_context(tc.tile_pool(name="stsb", bufs=2))
        pu = ctx.enter_context(tc.tile_pool(name="ucmp", bufs=3))
        pe_ = ctx.enter_context(tc.tile_pool(name="ecmp", bufs=3))
        psc = ctx.enter_context(tc.tile_pool(name="scal", bufs=3))
        pac = ctx.enter_context(tc.tile_pool(name="acmp", bufs=3))
        pat2 = ctx.enter_context(tc.tile_pool(name="atc", bufs=3))
        pat = ctx.enter_context(tc.tile_pool(name="atbd", bufs=3))
        pg = ctx.enter_context(tc.tile_pool(name="gsb", bufs=3))
        pem = ctx.enter_context(tc.tile_pool(name="embuf", bufs=3))
        pr = ctx.enter_context(tc.tile_pool(name="rbuf", bufs=3))
        pr3 = ctx.enter_context(tc.tile_pool(name="r3", bufs=2))

        ps_xt = ctx.enter_context(tc.tile_pool(name="ps_xt", bufs=1, space="PSUM"))
        ps_st = ctx.enter_context(tc.tile_pool(name="ps_st", bufs=1, space="PSUM"))
        ps_e = ctx.enter_context(tc.tile_pool(name="ps_e", bufs=1, space="PSUM"))
        ps_at1 = ctx.enter_context(tc.tile_pool(name="ps_at1", bufs=1, space="PSUM"))
        ps_at2 = ctx.enter_context(tc.tile_pool(name="ps_at2", bufs=1, space="PSUM"))
        ps_g = ctx.enter_context(tc.tile_pool(name="ps_g", bufs=1, space="PSUM"))
        ps_o = ctx.enter_context(tc.tile_pool(name="ps_o", bufs=2, space="PSUM"))

        # ---- load constants ----
        w12a = cpool.tile([F, F], BF16, tag="w12a")
        w2bb = cpool.tile([F, F], BF16, tag="w2bb")
        wa2c = cpool.tile([F, 1], BF16, tag="wa2c")
        w2brow = cpool.tile([1, F], BF16, tag="w2brow")
        identf = cpool.tile([RC, RC], FP32, tag="identf")
        identb = cpool.tile([RC, RC], BF16, tag="identb")
        wa1ones = cpool.tile([F, N_JOINTS], BF16, tag="wa1ones")
        lind = cpool.tile([G_CHUNK, F], BF16, tag="lind")
        sp = cpool.tile([NCH * N_JOINTS, NCH, F], BF16, tag="sp")
        camt = cpool.tile([RC, RC], BF16, tag="camt")
        for name, t in (("w12a", w12a), ("w2bb", w2bb), ("wa2c", wa2c),
                        ("w2brow", w2brow), ("identf", identf),
                        ("identb", identb), ("wa1ones", wa1ones),
                        ("lind", lind), ("camt", camt)):
            nc.sync.dma_start(t[:], cst[name][:])
        nc.sync.dma_start(sp[:], cst["sp"].rearrange("c k f -> k c f"))

        r0 = 0
        sli = 0
        sti = 0
        for slab_rows, sts in plan:
            sli += 1
            nfull = slab_rows // RC          # full 126-row chunk slots
            rem = slab_rows - nfull * RC     # tail rows in slot `nfull`
            nst = len(sts)

            x_sl = pxin.tile([RC, CH_SLAB, F], FP32, tag="x_sl")
            o_sl = pout.tile([RC, CH_SLAB, F], FP32, tag="o_sl")
            if nfull:
                nc.sync.dma_start(
                    x_sl[:, 0:nfull, :],
                    x_d[r0:r0 + nfull * RC, :].rearrange(
                        "(c i) f -> i c f", i=RC))
            if rem:
                nc.gpsimd.memset(x_sl[:, nfull, :], 0.0)
                nc.sync.dma_start(
                    x_sl[0:rem, nfull, :],
                    x_d[r0 + nfull * RC:r0 + slab_rows, :])

            # ---- pass 1 (whole slab): transpose x, compute t ----
            xt_sl = pxt.tile([F, CH_SLAB, F], BF16, tag="xt_sl")
            st_sb = pst.tile([1, G_CHUNK, CH_SLAB, N_JOINTS], BF16,
                             tag="st_sb")
            R3 = pr3.tile([G_CHUNK, CH_SLAB, N_JOINTS], BF16, tag="R3")
            if sli <= 2:
                nc.gpsimd.memset(xt_sl[:, :, RC:F], 0.0)
            if nst * NCH < CH_SLAB:
                # tail slab: zero the unused chunk slots once
                nc.gpsimd.memset(xt_sl[:, nst * NCH:CH_SLAB, 0:RC], 0.0)
                nc.gpsimd.memset(st_sb[:, :, nst * NCH:CH_SLAB, :], 0.0)
            for s, chunks in enumerate(sts):
                nch = len(chunks)
                c0 = s * NCH
                xt_ps = ps_xt.tile([F, NCH, F], FP32, tag="xt_ps")
                for c in range(nch):
                    nc.tensor.transpose(xt_ps[:, c, 0:RC],
                                        x_sl[:, c0 + c, :], identf[:])
                if nch < NCH:
                    nc.gpsimd.memset(xt_sl[:, c0 + nch:c0 + NCH, 0:RC], 0.0)
                nh = (nch + 1) // 2
                nc.vector.tensor_copy(xt_sl[:, c0:c0 + nh, 0:RC],
                                      xt_ps[:, 0:nh, 0:RC])
                if nch > nh:
                    nc.scalar.copy(xt_sl[:, c0 + nh:c0 + nch, 0:RC],
                                   xt_ps[:, nh:nch, 0:RC])

                # t in (q, c, jj) order: t[mate] = wa2 . x_row
                st_ps = ps_st.tile([1, G_CHUNK, NCH, N_JOINTS], FP32,
                                   tag="st_ps")
                nc.tensor.matmul(st_ps[:], wa2c[:], _perm_qcj(xt_sl, c0),
                                 start=True, stop=True)
                nc.vector.tensor_copy(st_sb[:, :, c0:c0 + NCH, :], st_ps[:])

            # one partition-scatter per slab: [1, (q, c, jj)] -> [q, c, jj]
            flat = st_sb[:]
            nc.gpsimd.dma_start(
                R3[:],
                bass.AP(flat.tensor, flat.offset,
                        [flat.ap[0], [1, G_CHUNK * CH_SLAB * N_JOINTS]]))

            # ---- pass 2 (per supertile): attention + output ----
            for s, chunks in enumerate(sts):
                nch = len(chunks)
                c0 = s * NCH
                sti += 1

                # -- compact e: e[i, (c, jj)] = s_i + t[mate] --
                # s-part: xt_c^T @ (wa1 (x) ones21); t-part: lind^T @ R3_c
                e_ps = ps_e.tile([F, NCH, N_JOINTS], FP32, tag="e_ps")
                for c in range(nch):
                    nc.tensor.matmul(e_ps[:, c, :], xt_sl[:, c0 + c, :],
                                     wa1ones[:], start=True, stop=False)
                    nc.tensor.matmul(e_ps[:, c, :], lind[:],
                                     R3[:, c0 + c, :],
                                     start=False, stop=True)

                # -- softmax (compact): prelu -> exp -> reduce -> recip --
                u = pu.tile([RC, NCH, N_JOINTS], FP32, tag="u")
                nc.scalar.activation(u[:, 0:nch, :], e_ps[0:RC, 0:nch, :],
                                     AF.Prelu, alpha=0.2)
                E = pe_.tile([RC, NCH, N_JOINTS], BF16, tag="E")
                nc.scalar.activation(E[:, 0:nch, :], u[:, 0:nch, :], AF.Exp)
                rowsum = psc.tile([RC, NCH], FP32, tag="rowsum")
                nc.vector.tensor_reduce(rowsum[:, 0:nch], E[:, 0:nch, :],
                                        mybir.AxisListType.X, ALU.add)
                rinv = psc.tile([RC, NCH], FP32, tag="rinv")
                nc.vector.reciprocal(rinv[:, 0:nch], rowsum[:, 0:nch])

                # -- beta_cmp = E * rinv --
                acmp = pac.tile([RC, NCH, N_JOINTS], BF16, tag="acmp")
                for c in range(nch):
                    nc.vector.tensor_scalar(
                        acmp[:, c, :], E[:, c, :], rinv[:, c:c + 1], None,
                        ALU.mult)

                # -- transpose compact beta: [84, 126] --
                at1_ps = ps_at1.tile([NCH * N_JOINTS, F], BF16, tag="at1_ps")
                nc.tensor.transpose(at1_ps[0:nch * N_JOINTS, 0:RC],
                                    acmp[:, 0:nch, :], identb[:])
                atc = pat2.tile([NCH * N_JOINTS, RC], BF16, tag="atc")
                nc.vector.tensor_copy(atc[0:nch * N_JOINTS, :],
                                      at1_ps[0:nch * N_JOINTS, 0:RC])

                # -- spread to block-diag (ungated), then gate*cam --
                at2_ps = ps_at2.tile([F, NCH, RC], FP32, tag="at2_ps")
                for c in range(nch):
                    nc.tensor.matmul(at2_ps[:, c, :],
                                     sp[0:nch * N_JOINTS, c, :],
                                     atc[0:nch * N_JOINTS, :],
                                     start=True, stop=True)
                at = pat.tile([F, NCH, F], BF16, tag="at")
                if sti <= 3:
                    # one-time: junk cols 0, bias row (126) = ones, row 127 = 0
                    nc.gpsimd.memset(at[0:RC, :, RC:F], 0.0)
                    nc.gpsimd.dma_start(at[RC:F, :, :], cst["atpad"][:])
                nc.vector.tensor_tensor(
                    at[0:RC, 0:nch, 0:RC], at2_ps[0:RC, 0:nch, :],
                    _bcast_c(camt[:], nch), ALU.mult)

                # -- g = x @ W12a (row-major, bf16); row 126 = W2_b --
                g_ps = ps_g.tile([F, NCH, F], FP32, tag="g_ps")
                for c in range(nch):
                    nc.tensor.matmul(g_ps[:, c, :], xt_sl[:, c0 + c, :],
                                     w12a[:], start=True, stop=True)
                g = pg.tile([F, NCH, F], BF16, tag="g")
                if sti <= 3:
                    nc.sync.dma_start(g[RC:RC + 1, :, :],
                                      _bcast_c(cst["w2brow"][:], NCH))
                nc.scalar.copy(g[0:RC, 0:nch, :], g_ps[0:RC, 0:nch, :])

                # -- o = at^T @ [g; W2_b] + x @ W2b  (bias via K=127) --
                o_ps = ps_o.tile([F, NCH, F], FP32, tag="o_ps")
                for c in range(nch):
                    nc.tensor.matmul(o_ps[:, c, :], at[0:RC + 1, c, :],
                                     g[0:RC + 1, c, :], start=True,
                                     stop=False)
                    nc.tensor.matmul(o_ps[:, c, :], xt_sl[:, c0 + c, :],
                                     w2bb[:], start=False, stop=True)

                # -- elu: em=exp(o), r=relu(o) (ACT); min(em-1, r) (POOL) --
                em = pem.tile([RC, NCH, F], BF16, tag="em")
                nc.scalar.activation(em[:, 0:nch, :], o_ps[0:RC, 0:nch, :],
                                     AF.Exp)
                rr = pr.tile([RC, NCH, F], BF16, tag="rr")
                nc.scalar.activation(rr[:, 0:nch, :], o_ps[0:RC, 0:nch, :],
                                     AF.Relu)
                nc.vector.scalar_tensor_tensor(
                    o_sl[:, c0:c0 + nch, :], em[:, 0:nch, :], -1.0,
                    rr[:, 0:nch, :], op0=ALU.add, op1=ALU.min)

            # -- store slab --
            if nfull:
                nc.sync.dma_start(
                    out_d[r0:r0 + nfull * RC, :].rearrange(
                        "(c i) f -> i c f", i=RC),
                    o_sl[:, 0:nfull, :])
            if rem:
                nc.sync.dma_start(
                    out_d[r0 + nfull * RC:r0 + slab_rows, :],
                    o_sl[0:rem, nfull, :])
            r0 += slab_rows


# ---------------------------------------------------------------------------
_PROG_CACHE = {}


def _get_program(rows):
    if rows not in _PROG_CACHE:
        _PROG_CACHE[rows] = build_program(rows)
    return _PROG_CACHE[rows]


def kernel(x, cam, W1, a, W2_w, W2_b):
    from concourse.bass_utils import run_bass_kernel_spmd

    x = np.ascontiguousarray(np.asarray(x, np.float32))
    consts = host_consts(cam, W1, a, W2_w, W2_b)
    nc = _get_program(ROWS_CORE)

    in_maps = []
    for core in range(N_CORES):
        m = {"x": x[core * ROWS_CORE:(core + 1) * ROWS_CORE]}
        m.update(consts)
        in_maps.append(m)
    res = run_bass_kernel_spmd(nc, in_maps, list(range(N_CORES)))
    out = np.concatenate([res.results[i]["out"] for i in range(N_CORES)], axis=0)
    return out.astype(np.float32)
